# revision 22
# baseline (speedup 1.0000x reference)
"""GATr->e Trainium2 kernel: 3 GAT blocks over a 100K-node/500K-edge graph.

Strategy: shard NODES across 8 cores. In each GAT block the x_e gather key
equals the segment (scatter) key, so a core that owns a node range and the
edges keyed into it never needs remote data -> zero collectives.

Host prep (per core, per key h/t):
  - nodes relabeled into 128-node tiles (snake-balanced by degree),
  - edges grouped by tile, padded to 128-edge groups (per-tile group counts
    G[j] maxed across cores so one program serves all 8 cores),
  - x_r rows shipped transposed in slot order (bf16 xs[128,S] + xb[65,S]
    with a ones-row), one-hot ST shipped as fp8 [128,S], local-dst columns
    (dc) shipped f32.

Device per tile j (128 nodes, G[j] groups of 128 edges):
  ns_all = (x_e * a_node).sum per 64-chunk            (DVE, batched per block)
  ns -> fp8 hi + fp8 (lo*16) columns                  (DVE)
  per group: e_r chunks -> wide PSUM [128, 132*3]     (PE; blocks 0+2 merged)
             nsg cols   = ST_g.T @ [hi|lo]            (PE, fp8)
  er0 -> SBUF bf16 (ACT), er2 -> resident store (Pool), rs -> (Pool)
  lg = nsg_hi + nsg_lo/16 + rs; lrelu; ex = exp       (DVE+ACT)
  per group: sp = onehot(dst)*ex (DVE 4x tensor_scalar)
             out_psum += sp.T @ er[.,0:65]            (PE, PSUM-accumulated)
  agg = out[:, :64] / (out[:,64]+1e-16); x_e += relu(agg)
Block 2 (h again) streams only ST: e_r2/rs2 come from the resident store.
Softmax max-subtraction is dropped: logits stay in [-10, 10] here and the
reference's +1e-16 guard is reproduced exactly on the s=0 (degree-0) case.
"""

import math
import numpy as np
import ml_dtypes

BF16 = ml_dtypes.bfloat16
FP8 = ml_dtypes.float8_e4m3

N_NODES = 100000
N_EDGES = 500000
E_HID = 64
IN_DIM = 192
NCORES = 8
NEG_SLOPE = 0.01
P = 128
CHUNK = 3          # erp PSUM chunk = 3 groups


class Cfg:
    def __init__(self, n_nodes=N_NODES, ncores=NCORES):
        self.n_nodes = n_nodes
        self.ncores = ncores
        self.npc = n_nodes // ncores            # nodes per core
        self.nbins = (self.npc + P - 1) // P    # 128-node tiles per core
        self.block_keys = [0, 1, 0]             # h, t, h


def _snake_bins(deg, nbins):
    """Deal nodes (sorted by degree desc) snake-wise into nbins bins."""
    order = np.argsort(-deg, kind="stable")
    n = len(order)
    rounds = (n + nbins - 1) // nbins
    fwd = np.arange(nbins, dtype=np.int32)
    seq = np.concatenate([fwd if r % 2 == 0 else fwd[::-1] for r in range(rounds)])
    bin_of = np.empty(n, dtype=np.int64)
    bin_of[order] = seq[:n]
    return bin_of


def _balance_bins(hd, td, nbins, kh, kt, iters=2000):
    """Assign nodes to bins so per-bin h/t loads fit caps: bins < kh (resp kt)
    get 6 groups (cap 768), the rest 5 (cap 640). Swap-refine from a snake
    start; G falls out of the actual loads, so non-convergence only pads."""
    caph = np.full(nbins, 5 * P, dtype=np.float64)
    caph[:kh] = 6 * P
    capt = np.full(nbins, 5 * P, dtype=np.float64)
    capt[:kt] = 6 * P
    bin_of = _snake_bins(hd + td, nbins)
    hl = np.bincount(bin_of, weights=hd, minlength=nbins)
    tl = np.bincount(bin_of, weights=td, minlength=nbins)
    for _ in range(iters):
        ovh = hl - caph
        ovt = tl - capt
        ov = np.maximum(ovh, ovt)
        A = int(np.argmax(ov))
        if ov[A] <= 0:
            break
        k_is_h = ovh[A] >= ovt[A]
        slack = (caph - hl) if k_is_h else (capt - tl)
        slack[A] = -1e9
        B = int(np.argmax(slack))
        ia = np.flatnonzero(bin_of == A)
        ib = np.flatnonzero(bin_of == B)
        dh = hd[ia][:, None] - hd[ib][None, :]
        dt = td[ia][:, None] - td[ib][None, :]
        cost = np.maximum.reduce([hl[A] - dh - caph[A], tl[A] - dt - capt[A],
                                  hl[B] + dh - caph[B], tl[B] + dt - capt[B]])
        qa, qb = np.unravel_index(np.argmin(cost), cost.shape)
        cur = max(ovh[A], ovt[A], hl[B] - caph[B], tl[B] - capt[B])
        if cost[qa, qb] >= cur:
            break
        a_i, b_i = ia[qa], ib[qb]
        bin_of[a_i], bin_of[b_i] = B, A
        hl[A] -= dh[qa, qb]
        tl[A] -= dt[qa, qb]
        hl[B] += dh[qa, qb]
        tl[B] += dt[qa, qb]
    return bin_of


def _host_prep(x_e, x_r, h, t, cfg):
    """Returns (per_core list of dicts of device arrays, G_prof, node_new)."""
    N, NC, NPC, NB = cfg.n_nodes, cfg.ncores, cfg.npc, cfg.nbins
    hdeg = np.bincount(h, minlength=N).astype(np.float64)
    tdeg = np.bincount(t, minlength=N).astype(np.float64)

    kh = kt = 0
    for c in range(NC):
        lo = c * NPC
        kh = max(kh, int(np.ceil(max(0.0, hdeg[lo:lo + NPC].sum() - 5 * P * NB) / P)))
        kt = max(kt, int(np.ceil(max(0.0, tdeg[lo:lo + NPC].sum() - 5 * P * NB) / P)))

    node_new = np.empty(N, dtype=np.int64)  # old -> new local id (within core)
    for c in range(NC):
        lo = c * NPC
        nb = _balance_bins(hdeg[lo:lo + NPC], tdeg[lo:lo + NPC], NB, kh, kt)
        order = np.argsort(nb, kind="stable")
        counts = np.bincount(nb, minlength=NB)
        starts = np.concatenate(([0], np.cumsum(counts)))[:NB]
        newlocal = np.empty(NPC, dtype=np.int64)
        newlocal[order] = np.arange(NPC) - starts[nb[order]] + nb[order] * P
        node_new[lo:lo + NPC] = newlocal

    x_r_b = np.ascontiguousarray(x_r).astype(BF16)

    per_core = [dict() for _ in range(NC)]
    G_prof = {}
    for kname, key in (("h", h), ("t", t)):
        kc = key // NPC
        knew = node_new[key]
        kbin = knew // P
        loads = np.zeros((NC, NB), dtype=np.int64)
        np.add.at(loads, (kc, kbin), 1)
        G = ((loads + P - 1) // P).max(axis=0).astype(np.int64)
        G_prof[kname] = G
        off = P * np.concatenate(([0], np.cumsum(G)))
        S_tot = int(off[-1])
        for c in range(NC):
            ec = np.flatnonzero(kc == c)
            be = kbin[ec]
            dle = (knew[ec] % P).astype(np.int64)
            eo = np.argsort(be, kind="stable")
            be_s, dle_s, eid = be[eo], dle[eo], ec[eo]
            cnt = np.bincount(be_s, minlength=NB)
            bstart = np.concatenate(([0], np.cumsum(cnt)))[:NB]
            slots = off[be_s] + (np.arange(len(eo)) - bstart[be_s])

            rows = np.zeros((S_tot, 193), dtype=BF16)
            rows[slots, :192] = x_r_b[eid]
            rows[slots, 192] = 1
            rT = np.ascontiguousarray(rows.T)
            st8 = np.zeros((P, S_tot), dtype=FP8)
            st8[dle_s, slots] = 1
            dc = np.full(S_tot, -1.0, dtype=np.float32)
            dc[slots] = dle_s
            dc = np.ascontiguousarray(dc.reshape(-1, P).T)
            d = per_core[c]
            d["xs_" + kname] = np.ascontiguousarray(rT[:128])
            d["xb_" + kname] = np.ascontiguousarray(rT[128:193])
            d["st_" + kname] = st8
            d["dc_" + kname] = dc

    for c in range(NC):
        lo = c * NPC
        xe_dev = np.zeros((NB * P, E_HID), dtype=np.float32)
        xe_dev[node_new[lo:lo + NPC]] = x_e[lo:lo + NPC]
        # [NB*P, 64] -> [128, NB*64] partition-major so the DMA is contiguous
        per_core[c]["xe"] = np.ascontiguousarray(
            xe_dev.reshape(NB, P, E_HID).transpose(1, 0, 2).reshape(P, NB * E_HID))
    return per_core, G_prof, node_new


def _weights_arrays(Wr, br, Wr1, br1, Wr2, br2, ah, ah1, at, ar1, ar2, ar3):
    """cbf bf16 [128, 524]: wa_w[0:132] | wb_w rows0:65 [132:264] | wa1[264:330]
    | wb1 rows0:65 [330:396] | iota [396:524].
    cf32 [128, 192 + GT_h + GT_t]: a-vecs then dc_h, dc_t appended by caller."""
    def block_cols(W, bias, a_rel):
        wa = np.zeros((128, 66), dtype=np.float32)
        wb = np.zeros((65, 66), dtype=np.float32)
        WT = W.T  # [192, 64]
        wa[:, 0:64] = WT[0:128]
        wa[:, 65] = (WT @ a_rel)[0:128]
        wb[0:64, 0:64] = WT[128:192]
        wb[0:64, 65] = (WT @ a_rel)[128:192]
        wb[64, 0:64] = bias
        wb[64, 64] = 1.0
        wb[64, 65] = float(bias @ a_rel)
        return wa, wb

    wa0, wb0 = block_cols(Wr, br, ar1)
    wa2, wb2 = block_cols(Wr2, br2, ar3)
    wa1, wb1 = block_cols(Wr1, br1, ar2)
    cbf = np.zeros((128, 524), dtype=BF16)
    cbf[:, 0:66] = wa0.astype(BF16)
    cbf[:, 66:132] = wa2.astype(BF16)
    cbf[0:65, 132:198] = wb0.astype(BF16)
    cbf[0:65, 198:264] = wb2.astype(BF16)
    cbf[:, 264:330] = wa1.astype(BF16)
    cbf[0:65, 330:396] = wb1.astype(BF16)
    cbf[:, 396:524] = np.tile(np.arange(128, dtype=np.float32)[None, :],
                              (128, 1)).astype(BF16)
    abc = np.zeros((128, 192), dtype=np.float32)
    for i, a_node in enumerate((ah, at, ah1)):
        abc[:, i * 64:(i + 1) * 64] = np.tile(a_node[None, :], (128, 1))
    return cbf, abc


def build_program(cfg, G_prof, _prune=True):
    import sys
    if "/opt/trn_rl_repo" not in sys.path:
        sys.path.insert(0, "/opt/trn_rl_repo")
    from concourse import bass, mybir, tile
    from concourse.vector_clock import ScopedClock

    if not getattr(tile.TileContext, "_ant_split_drain", False):
        _orig_dab = tile.TileContext._drain_and_barrier

        def _split_dab(self, tick_clock, wait_clock):
            nc_ = self.nc
            drain_inst = nc_.sync.drain()
            wait_clock.add_sem_waits(
                drain_inst.ins, ScopedClock({None: tick_clock.global_clock})
            )
            si = drain_inst.ins.sync_info
            waits = list(si.on_wait) if si and si.on_wait else []
            if len(waits) > 1:
                upd = list(si.on_update) if si.on_update else []
                drain_inst.ins.sync_info = mybir.SyncInfo(on_wait=waits[:1], on_update=upd)
                for w in waits[1:]:
                    d2 = nc_.sync.drain()
                    d2.ins.sync_info = mybir.SyncInfo(on_wait=[w], on_update=[])
            nc_.all_engine_barrier()
            assert self.sems is not None
            popped = nc_._tile_sem_poison_stack.pop()
            assert popped is self._sem_poison
            nc_.clear_and_free_semaphores(list(self.sems.allocated().values()))
            nc_.all_engine_barrier()

        tile.TileContext._drain_and_barrier = _split_dab
        tile.TileContext._ant_split_drain = True

    NB = cfg.nbins
    nc = bass.Bass(enable_partition_id=False)
    f32, bf = mybir.dt.float32, mybir.dt.bfloat16
    f8 = mybir.dt.float8e4
    A = mybir.AluOpType
    AF = mybir.ActivationFunctionType
    AP = bass.AP

    G_h, G_t = G_prof["h"], G_prof["t"]
    GT = {"h": int(G_h.sum()), "t": int(G_t.sum())}
    off = {kn: P * np.concatenate(([0], np.cumsum(G_prof[kn]))) for kn in ("h", "t")}
    S = {kn: int(off[kn][-1]) for kn in ("h", "t")}

    dram = {}
    for kn in ("h", "t"):
        dram["xs_" + kn] = nc.dram_tensor("xs_" + kn, [128, S[kn]], bf, kind="ExternalInput")
        dram["xb_" + kn] = nc.dram_tensor("xb_" + kn, [65, S[kn]], bf, kind="ExternalInput")
        dram["st_" + kn] = nc.dram_tensor("st_" + kn, [128, S[kn]], f8, kind="ExternalInput")
    dram["cbf"] = nc.dram_tensor("cbf", [128, 524], bf, kind="ExternalInput")
    dram["cf32"] = nc.dram_tensor("cf32", [128, 192], f32, kind="ExternalInput")
    dram["dcb"] = nc.dram_tensor("dcb", [128, GT["h"] + GT["t"]], f32, kind="ExternalInput")
    dram["xe"] = nc.dram_tensor("xe", [128, NB * E_HID], f32, kind="ExternalInput")
    xe_out = nc.dram_tensor("xe_out", [128, NB * E_HID], f32, kind="ExternalOutput")
    dcbase = {"h": 0, "t": GT["h"]}

    def strided(tileap, off0, bstride, n, w=None):
        """AP over a tile: partitions x [n blocks of w (or scalar) at bstride]."""
        base = tileap[:]
        pstride = base.ap[0][0]
        if w is None:
            return AP(tileap.tensor, base.offset + off0, [[pstride, 128], [bstride, n]])
        return AP(tileap.tensor, base.offset + off0,
                  [[pstride, 128], [bstride, n], [1, w]])

    with tile.TileContext(nc) as tc:
        with (
            tc.tile_pool(name="const", bufs=1) as cpool,
            tc.tile_pool(name="ld", bufs=3) as ld,
            tc.tile_pool(name="work", bufs=4) as work,
            tc.tile_pool(name="ns", bufs=2) as nspool,
            tc.tile_pool(name="er0", bufs=3) as er0pool,
            tc.tile_pool(name="spool", bufs=6) as spool,
            tc.tile_pool(name="sppool", bufs=10) as sppool,
            tc.tile_pool(name="erps", bufs=3, space="PSUM") as erps_pool,
            tc.tile_pool(name="nsgps", bufs=2, space="PSUM") as nsgps_pool,
            tc.tile_pool(name="outps", bufs=2, space="PSUM") as outps_pool,
        ):
            cbf_sb = cpool.tile([128, 524], bf)
            cf_sb = cpool.tile([128, 192], f32)
            dcb_sb = cpool.tile([128, GT["h"] + GT["t"]], f32)
            xe_sb = cpool.tile([128, NB * E_HID], f32)
            scr = cpool.tile([128, NB * E_HID], f32)
            rl_all = cpool.tile([128, NB * E_HID], f32)
            er2_st = cpool.tile([128, GT["h"] * 65], bf)
            rs02_st = cpool.tile([128, 2 * GT["h"]], f32)
            nc.sync.dma_start(out=cbf_sb[:], in_=dram["cbf"][:])
            nc.sync.dma_start(out=cf_sb[:], in_=dram["cf32"][:])
            nc.sync.dma_start(out=dcb_sb[:], in_=dram["dcb"][:])
            nc.sync.dma_start(out=xe_sb[:], in_=dram["xe"][:])

            def wa_ap(b):  # rhs for the 128-row chunk
                if b == 0:
                    return cbf_sb[:, 0:132]
                return cbf_sb[:, 264:330]

            def wb_ap(b):  # rhs for the 65-row chunk
                if b == 0:
                    return cbf_sb[0:65, 132:264]
                return cbf_sb[0:65, 330:396]

            iota_ap = cbf_sb[:, 396:524]

            # warmup ops observe each const DMA once per engine, so no later
            # compute instruction needs more than one fresh sync wait
            wup = outps_pool.tile([128, 65], f32, tag="outp", name="wup")
            nc.tensor.matmul(wup[0:1, 0:1], iota_ap[:, 0:1], cbf_sb[:, 0:1],
                             start=True, stop=True, skip_group_check=True)
            wupv = work.tile([1, 4], f32, tag="wupv", name="wupv")
            nc.vector.tensor_copy(wupv[:, 0:1], cf_sb[0:1, 0:1])
            nc.vector.tensor_copy(wupv[:, 1:2], xe_sb[0:1, 0:1])
            nc.vector.tensor_copy(wupv[:, 2:3], cbf_sb[0:1, 0:1])
            nc.vector.tensor_copy(wupv[:, 3:4], dcb_sb[0:1, 0:1])
            wupa = work.tile([1, 3], f32, tag="wupa", name="wupa")
            nc.scalar.activation(wupa[:, 0:1], cbf_sb[0:1, 0:1], AF.Copy)
            nc.scalar.activation(wupa[:, 1:2], cf_sb[0:1, 0:1], AF.Copy)
            nc.scalar.activation(wupa[:, 2:3], xe_sb[0:1, 0:1], AF.Copy)
            wupp = work.tile([1, 4], f32, tag="wupp", name="wupp")
            nc.gpsimd.tensor_copy(wupp[:, 0:1], cbf_sb[0:1, 0:1])
            nc.gpsimd.tensor_copy(wupp[:, 1:2], cf_sb[0:1, 0:1])
            nc.gpsimd.tensor_copy(wupp[:, 2:3], xe_sb[0:1, 0:1])
            nc.gpsimd.tensor_copy(wupp[:, 3:4], dcb_sb[0:1, 0:1])

            for b in range(3):
                kn = ["h", "t"][cfg.block_keys[b]]
                G = G_prof[kn]
                rhs_w = 132 if b == 0 else 66

                # ---- batched node scores for this block -> fp8 hi/lo ----
                a_b = AP(cf_sb.tensor, cf_sb[:].offset + b * 64,
                         [[cf_sb[:].ap[0][0], 128], [0, NB], [1, 64]])
                xe3 = AP(xe_sb.tensor, xe_sb[:].offset,
                         [[xe_sb[:].ap[0][0], 128], [64, NB], [1, 64]])
                sc3 = AP(scr.tensor, scr[:].offset,
                         [[scr[:].ap[0][0], 128], [64, NB], [1, 64]])
                nc.vector.tensor_tensor(sc3, xe3, a_b, op=A.mult)
                ns_f = nspool.tile([128, NB], f32, tag="nsf", name="nsf")
                nc.vector.tensor_reduce(ns_f[:], sc3, axis=mybir.AxisListType.X,
                                        op=A.add)
                ns8 = nspool.tile([128, 2 * NB], f8, tag="ns8", name="ns8")
                hi_ap = strided(ns8, 0, 2, NB)
                lo_ap = strided(ns8, 1, 2, NB)
                nc.vector.tensor_copy(hi_ap, ns_f[:])
                hif = nspool.tile([128, NB], f32, tag="hif", name="hif")
                nc.vector.tensor_copy(hif[:], hi_ap)
                lof = nspool.tile([128, NB], f32, tag="lof", name="lof")
                nc.vector.tensor_tensor(lof[:], ns_f[:], hif[:], op=A.subtract)
                nc.vector.tensor_scalar_mul(lo_ap, lof[:], 16.0)

                state = {}
                pend = {}
                NPAIR = (NB + 1) // 2

                def dma_pair(p, b=b, kn=kn, pend=pend):
                    j0 = 2 * p
                    j1 = min(2 * p + 1, NB - 1)
                    base = int(off[kn][j0])
                    W = int(off[kn][j1 + 1]) - base
                    st8 = ld.tile([128, W], f8, tag="st", name="st")
                    nc.sync.dma_start(out=st8[:],
                                      in_=dram["st_" + kn][:, base:base + W])
                    if b < 2:
                        xs = ld.tile([128, W], bf, tag="xs", name="xs")
                        xb = ld.tile([65, W], bf, tag="xb", name="xb")
                        nc.sync.dma_start(out=xs[:],
                                          in_=dram["xs_" + kn][:, base:base + W])
                        nc.sync.dma_start(out=xb[:],
                                          in_=dram["xb_" + kn][:, base:base + W])
                    else:
                        xs = xb = None
                    pend[p] = (st8, xs, xb, base)

                def front(j, b=b, kn=kn, G=G, rhs_w=rhs_w, ns8=ns8, state=state,
                          pend=pend):
                    Gj = int(G[j])
                    base = int(off[kn][j])
                    gbase = base // P
                    st8, xs, xb, pbase = pend[j // 2]
                    c_off = base - pbase
                    erps = []
                    if b < 2:
                        for c0 in range(0, Gj, CHUNK):
                            n = min(CHUNK, Gj - c0)
                            erp = erps_pool.tile([128, n * rhs_w], f32, tag="erp",
                                                 name="erp")
                            for g in range(c0, c0 + n):
                                sl = slice(c_off + g * P, c_off + (g + 1) * P)
                                osl = slice((g - c0) * rhs_w, (g - c0 + 1) * rhs_w)
                                nc.tensor.matmul(erp[:, osl], xs[:, sl], wa_ap(b),
                                                 start=True, stop=False,
                                                 skip_group_check=True)
                                nc.tensor.matmul(erp[:, osl], xb[:, sl], wb_ap(b),
                                                 start=False, stop=True,
                                                 skip_group_check=True)
                            erps.append((erp, c0, n))
                    nsg = nsgps_pool.tile([128, 2 * Gj], f32, tag="nsg", name="nsg")
                    for g in range(Gj):
                        sl = slice(c_off + g * P, c_off + (g + 1) * P)
                        nc.tensor.matmul(nsg[:, 2 * g:2 * g + 2], st8[:, sl],
                                         ns8[:, 2 * j:2 * j + 2], start=True,
                                         stop=True, skip_group_check=True)
                    # copies out of the wide PSUM chunks (PSUM readers must be
                    # ACT or DVE - GPSIMD cannot access PSUM on hardware)
                    if b == 0:
                        er0 = er0pool.tile([128, 65 * Gj], bf, tag="er0", name="er0")
                        for erp, c0, n in erps:
                            nc.scalar.activation(
                                strided(er0, 65 * c0, 65, n, 65),
                                strided(erp, 0, rhs_w, n, 65), AF.Copy)
                            nc.scalar.activation(
                                strided(er2_st, (gbase + c0) * 65, 65, n, 65),
                                strided(erp, 66, rhs_w, n, 65), AF.Copy)
                            # rs0+rs2 pair in one strided copy
                            nc.vector.tensor_copy(
                                AP(rs02_st.tensor,
                                   rs02_st[:].offset + 2 * (gbase + c0),
                                   [[rs02_st[:].ap[0][0], 128], [2, n], [1, 2]]),
                                AP(erp.tensor, erp[:].offset + 65,
                                   [[erp[:].ap[0][0], 128], [rhs_w, n], [66, 2]]))
                        rs_in = strided(rs02_st, 2 * gbase, 2, Gj)
                    elif b == 1:
                        er0 = er0pool.tile([128, 65 * Gj], bf, tag="er0", name="er0")
                        rs0 = work.tile([128, Gj], f32, tag="rs0", name="rs0")
                        for erp, c0, n in erps:
                            nc.scalar.activation(
                                strided(er0, 65 * c0, 65, n, 65),
                                strided(erp, 0, rhs_w, n, 65), AF.Copy)
                            nc.vector.tensor_copy(
                                strided(rs0, c0, 1, n),
                                strided(erp, 65, rhs_w, n))
                        rs_in = rs0[:]
                    else:
                        er0 = None
                        rs_in = strided(rs02_st, 2 * gbase + 1, 2, Gj)
                    # logits -> ex  (only one PSUM operand per instruction)
                    t1 = work.tile([128, Gj], f32, tag="t1", name="t1")
                    nc.vector.scalar_tensor_tensor(
                        t1[:], strided(nsg, 1, 2, Gj), 1.0 / 16.0,
                        rs_in, op0=A.mult, op1=A.add)
                    lg = work.tile([128, Gj], f32, tag="lg", name="lg")
                    nc.vector.tensor_tensor(lg[:], strided(nsg, 0, 2, Gj), t1[:],
                                            op=A.add)
                    lr = work.tile([128, Gj], f32, tag="lr", name="lr")
                    nc.vector.scalar_tensor_tensor(lr[:], lg[:], NEG_SLOPE, lg[:],
                                                   op0=A.mult, op1=A.max)
                    ex = work.tile([128, Gj], f32, tag="ex", name="ex")
                    nc.scalar.activation(ex[:], lr[:], AF.Exp)
                    state[j] = (ex, er0, gbase)

                def back(j, b=b, kn=kn, G=G, state=state):
                    Gj = int(G[j])
                    ex, er0, gbase = state.pop(j)
                    outp = outps_pool.tile([128, 65], f32, tag="outp", name="outp")
                    nd = max(1, Gj - 2)  # first groups' sp on DVE, last 2 on Pool
                    for g in range(Gj):
                        col = dcbase[kn] + gbase + g
                        dccol = dcb_sb[:, col:col + 1]
                        if g < nd:
                            sp = spool.tile([128, 128], bf, tag="sp", name="sp")
                            nc.vector.tensor_scalar(sp[:], iota_ap, scalar1=dccol,
                                                    scalar2=ex[:, g:g + 1],
                                                    op0=A.is_equal, op1=A.mult)
                        else:
                            sp = sppool.tile([128, 128], bf, tag="spp", name="spp")
                            nc.gpsimd.tensor_scalar(sp[:], iota_ap, scalar1=dccol,
                                                    scalar2=ex[:, g:g + 1],
                                                    op0=A.is_equal, op1=A.mult)
                        if er0 is not None:
                            rhs = er0[:, g * 65:(g + 1) * 65]
                        else:
                            rhs = er2_st[:, (gbase + g) * 65:(gbase + g + 1) * 65]
                        nc.tensor.matmul(outp[:], sp[:], rhs, start=(g == 0),
                                         stop=(g == Gj - 1), skip_group_check=True)
                    s_eps = work.tile([128, 1], f32, tag="seps", name="seps")
                    nc.vector.tensor_scalar_add(s_eps[:], outp[:, 64:65], 1e-16)
                    rec = work.tile([128, 1], f32, tag="rec", name="rec")
                    nc.vector.reciprocal(rec[:], s_eps[:])
                    nc.scalar.activation(rl_all[:, j * 64:(j + 1) * 64],
                                         outp[:, 0:64], AF.Relu, scale=rec[:])

                dma_pair(0)
                if NPAIR > 1:
                    dma_pair(1)
                for j in range(NB):
                    if j % 2 == 0 and j // 2 + 2 < NPAIR:
                        dma_pair(j // 2 + 2)
                    front(j)
                    if j >= 1:
                        back(j - 1)
                back(NB - 1)

                nc.gpsimd.tensor_tensor(xe_sb[:], xe_sb[:], rl_all[:], op=A.add)

            nc.sync.dma_start(out=xe_out[:], in_=xe_sb[:])
    if _prune:
        _fix_sync_waits(nc, mybir)
        _pad_multiwait_dmas(nc, mybir)
    return nc, dram


def _pad_multiwait_dmas(nc, mybir):
    """Walrus allows one sync-wait slot per instruction. After transitive
    pruning some DMA triggers still carry 2 waits (the queue round-robin
    interleaves streams whose producers don't cover each other). Since the
    SP queue executes in order, move the extra waits onto wait-only SP nops
    spliced immediately before the DMA - semantically identical."""
    targets = []
    for bb in nc.m.functions[0].blocks:
        for i, inst in enumerate(bb.instructions):
            si = inst.sync_info
            waits = list(si.on_wait) if si and si.on_wait else []
            if len(waits) <= 1 or type(inst).__name__ == "InstDrain":
                continue
            assert str(inst.engine) == "EngineType.SP", (
                f"multi-wait non-SP instruction {inst.name} {type(inst).__name__}")
            targets.append((bb, i, inst, waits))
    for bb, i, inst, waits in reversed(targets):
        upd = list(inst.sync_info.on_update) if inst.sync_info.on_update else []
        inst.sync_info = mybir.SyncInfo(on_wait=waits[:1], on_update=upd)
        pads = []
        for w in waits[1:]:
            pad = nc.sync.nop(nofuse=True)
            pad.ins.sync_info = mybir.SyncInfo(on_wait=[w], on_update=[])
            # nop() appended the instruction to the tail of some block; move it
            for bb2 in nc.m.functions[0].blocks:
                for k in range(len(bb2.instructions) - 1, -1, -1):
                    if bb2.instructions[k] is pad.ins:
                        bb2.instructions.pop(k)
                        break
            pads.append(pad.ins)
        for p in reversed(pads):
            bb.instructions.insert(i, p)


def _fix_sync_waits(nc, mybir):
    """Walrus here allows only ONE sync-wait slot per TPB compute instruction.
    Prune redundant waits via vector-clock transitivity: each instruction's
    observed clock = its engine's running clock + the observed clocks of the
    producers of its waits. A wait already implied by the other kept waits
    (or by the engine clock) is dropped. Own-engine waits fall out for free."""
    import bisect
    sem_hist = {}      # sem -> ([cum values], [inst idx])
    sem_cum = {}
    snap = []          # idx -> observed clock AFTER retire
    eng_obs = {}
    leftover = []

    def merge(dst, src):
        for s, v in src.items():
            if dst.get(s, -1) < v:
                dst[s] = v

    idx = 0
    for bb in nc.m.functions[0].blocks:
        for inst in bb.instructions:
            si = inst.sync_info
            eng = str(inst.engine)
            obs = eng_obs.setdefault(eng, {})
            waits = list(si.on_wait) if si and si.on_wait else []
            covs, prods, simple = [], [], True
            for w in waits:
                if str(w.wait_mode) != "sem-ge-imm" or w.sync_type != "semaphore":
                    simple = False
                    covs.append({}); prods.append(-1)
                    continue
                s, v = str(w.ant_name), w.wait_value
                hist = sem_hist.get(s)
                p = -1
                if hist is not None:
                    q = bisect.bisect_left(hist[0], v)
                    if q < len(hist[0]):
                        p = hist[1][q]
                covs.append(dict(snap[p]) if p >= 0 else {s: v})
                if p >= 0 and covs[-1].get(s, -1) < v:
                    covs[-1][s] = v
                prods.append(p)
            tname = type(inst).__name__
            if simple and len(waits) > 1 and tname != "InstDrain":
                order = sorted(range(len(waits)), key=lambda q2: -prods[q2])
                combined = dict(obs)
                keep = []
                for q2 in order:
                    w = waits[q2]
                    s, v = str(w.ant_name), w.wait_value
                    if combined.get(s, -1) >= v:
                        continue
                    keep.append(w)
                    merge(combined, covs[q2])
                if len(keep) > 1 and tname != "InstDMACopy":
                    leftover.append((inst.name, tname, eng,
                                     [str(w)[:70] for w in keep]))
                upd = list(si.on_update) if si.on_update else []
                inst.sync_info = mybir.SyncInfo(on_wait=keep, on_update=upd)
            for c in covs:
                merge(obs, c)
            if si and si.on_update:
                for u in si.on_update:
                    s = str(u.ant_name)
                    if str(u.update_mode) != "sem-inc":
                        sem_hist.pop(s, None)
                        continue
                    cum = sem_cum.get(s, 0) + (u.update_value or 1)
                    sem_cum[s] = cum
                    h2 = sem_hist.setdefault(s, ([], []))
                    h2[0].append(cum)
                    h2[1].append(idx)
                    if obs.get(s, -1) < cum:
                        obs[s] = cum
            snap.append(dict(obs))
            idx += 1
    assert not leftover, f"unpruned multi-wait instrs (n={len(leftover)}): {leftover[:4]}"


def _run(nc, in_maps, ncores, trace=False):
    import sys
    if "/opt/trn_rl_repo" not in sys.path:
        sys.path.insert(0, "/opt/trn_rl_repo")
    from concourse.bass_utils import run_bass_kernel_spmd
    return run_bass_kernel_spmd(nc, in_maps, list(range(ncores)), trace=trace)


def timed_run(nc, in_maps, ncores, iters=6):
    """Time pure device execution: jit without donation, device-resident inputs."""
    import sys, time
    if "/opt/trn_rl_repo" not in sys.path:
        sys.path.insert(0, "/opt/trn_rl_repo")
    import jax
    import numpy as _np
    from concourse import bass2jax, mybir
    from concourse.bass2jax import _bass_exec_p, install_neuronx_cc_hook
    from jax.sharding import Mesh, PartitionSpec, NamedSharding
    from jax.experimental.shard_map import shard_map
    install_neuronx_cc_hook()
    assert nc.partition_id_tensor is None and nc.dbg_addr is None
    in_names, out_names, out_avals, zero_outs = [], [], [], []
    for alloc in nc.m.functions[0].allocations:
        if not isinstance(alloc, mybir.MemoryLocationSet):
            continue
        name = alloc.memorylocations[0].name
        if alloc.kind == "ExternalInput":
            in_names.append(name)
        elif alloc.kind == "ExternalOutput":
            shape = tuple(alloc.tensor_shape)
            dtype = mybir.dt.np(alloc.dtype)
            out_names.append(name)
            out_avals.append(jax.core.ShapedArray(shape, dtype))
            zero_outs.append(_np.zeros(shape, dtype))
    n_params = len(in_names)
    all_names = in_names + out_names

    def _body(*args):
        outs = _bass_exec_p.bind(
            *args, out_avals=tuple(out_avals), in_names=tuple(all_names),
            out_names=tuple(out_names), lowering_input_output_aliases=(),
            sim_require_finite=True, sim_require_nnan=True, nc=nc)
        return tuple(outs)

    devices = jax.devices()[:ncores]
    mesh = Mesh(_np.asarray(devices), ("core",))
    nsh = NamedSharding(mesh, PartitionSpec("core"))
    in_specs = (PartitionSpec("core"),) * (n_params + len(out_names))
    out_specs = (PartitionSpec("core"),) * len(out_names)
    fn = jax.jit(shard_map(_body, mesh=mesh, in_specs=in_specs,
                           out_specs=out_specs, check_rep=False), keep_unused=True)
    concat = [jax.device_put(_np.concatenate([_np.asarray(in_maps[c][n])
                                              for c in range(ncores)], axis=0), nsh)
              for n in in_names]
    concat += [jax.device_put(_np.concatenate([z] * ncores, axis=0), nsh)
               for z in zero_outs]
    r = fn(*concat)
    jax.block_until_ready(r)
    times = []
    for _ in range(iters):
        t0 = time.perf_counter()
        r = fn(*concat)
        jax.block_until_ready(r)
        times.append(time.perf_counter() - t0)
    return times


def kernel(x_e, x_r, edge_index, rel_size, Wr, br, Wr1, br1, Wr2, br2,
           ah, ah1, at, ar1, ar2, ar3, _trace=False, _cfg=None):
    cfg = _cfg or Cfg()
    x_e = np.asarray(x_e, np.float32)
    x_r = np.asarray(x_r, np.float32)
    ei = np.asarray(edge_index)
    h = ei[0].astype(np.int64)
    t = ei[1].astype(np.int64)
    rs_idx = np.asarray(rel_size).astype(np.int64)
    if not np.array_equal(rs_idx, np.arange(len(rs_idx), dtype=np.int64)):
        x_r = np.ascontiguousarray(np.asarray(x_r)[rs_idx])

    per_core, G_prof, node_new = _host_prep(x_e, x_r, h, t, cfg)
    cbf, abc = _weights_arrays(
        np.asarray(Wr, np.float32), np.asarray(br, np.float32),
        np.asarray(Wr1, np.float32), np.asarray(br1, np.float32),
        np.asarray(Wr2, np.float32), np.asarray(br2, np.float32),
        np.asarray(ah, np.float32), np.asarray(ah1, np.float32),
        np.asarray(at, np.float32), np.asarray(ar1, np.float32),
        np.asarray(ar2, np.float32), np.asarray(ar3, np.float32))

    nc, _ = build_program(cfg, G_prof)
    in_maps = []
    for c in range(cfg.ncores):
        pc = per_core[c]
        m = {"xe": pc["xe"], "cbf": cbf, "cf32": abc}
        m["dcb"] = np.ascontiguousarray(
            np.concatenate([pc["dc_h"], pc["dc_t"]], axis=1))
        for kn in ("h", "t"):
            m["xs_" + kn] = pc["xs_" + kn]
            m["xb_" + kn] = pc["xb_" + kn]
            m["st_" + kn] = pc["st_" + kn]
        in_maps.append(m)
    kernel._last_nc = nc
    kernel._last_in_maps = in_maps
    res = _run(nc, in_maps, cfg.ncores, trace=_trace)

    out = np.empty((cfg.n_nodes, E_HID), dtype=np.float32)
    NPC, NB = cfg.npc, cfg.nbins
    for c in range(cfg.ncores):
        dev = np.asarray(res.results[c]["xe_out"], np.float32)
        # [128, NB*64] -> [NB*128, 64]
        dev = dev.reshape(128, NB, E_HID).transpose(1, 0, 2).reshape(NB * P, E_HID)
        lo = c * NPC
        out[lo:lo + NPC] = dev[node_new[lo:lo + NPC]]
    if _trace:
        kernel._last_result = res
    return out
